# revision 4
# baseline (speedup 1.0000x reference)
"""Trainium2 Bass kernel for the batched attention module:

    proj   = input @ W.T + b            # [B, TQ, 2H]
    scores = proj @ enc                 # [B, TQ, S]   (enc: [B, 2H, S], S == 2H)
    attn   = softmax(scores, axis=-1)
    out    = attn @ enc                 # [B, TQ, S]

Sharding: data-parallel over batch, one batch per NeuronCore (8 cores).

Design (measured ~320us on HW, vs 403us for the all-f32r grouped version):
  - 16-bit path in fp16 (NOT bf16): same 1-cycle/row PE rate, 8x finer
    mantissa. bf16 scores noise flips near-tied softmax argmaxes and
    landed at rel-err 1.9e-2 vs the 2e-2 gate; fp16 sits at 2.9e-3.
    All tensors are host-converted to fp16 (values are O(1), well within
    fp16 range); P1 accumulates in fp32 PSUM, softmax stats in fp32.
  - enc resident in SBUF as one fp16 tile (64KB/partition); W and the
    full projT also resident, so every DRAM byte is read once (14MB
    total vs 44MB for the baseline).
  - P1 (proj) runs dt-outer in the front while wt/inp/enc stream in on
    two HWDGE queues (sync + scalar), ordered by first use.
  - scores run c-outer into FOUR separate PSUM chunk tiles: each
    512-chunk completes early, its row-max runs on DVE during the next
    chunk, and (tile-granular WAR) chunk c+1's accumulation start does
    not wait on chunk c's readers.
  - softmax: chunk maxes -> combined negated max -> chunked Exp on ACT
    (fp16 out, fp32 accum per chunk, partial sums combined) ->
    reciprocal; per-chunk transposes (fp16, 2x faster on PE) follow
    each exp chunk.
  - PE order per q-tile: out(qt-1) | transp(qt) | scores(qt+1), so the
    softmax latency of tile qt hides entirely under out(qt-1); the
    steady loop runs gap-free on the PE.
"""

import sys

import numpy as np

for _p in ("/opt/trn_rl_repo",):
    if _p not in sys.path:
        sys.path.insert(0, _p)

from concourse import bacc, bass, mybir, tile  # noqa: E402
from concourse.bass_utils import run_bass_kernel_spmd  # noqa: E402
from concourse.masks import make_identity  # noqa: E402

F32 = mybir.dt.float32
FP16 = mybir.dt.float16
AF = mybir.ActivationFunctionType
AX = mybir.AxisListType
ALU = mybir.AluOpType

B = 8
TQ = 1024
H = 1024
D = 2 * H  # 2048 contraction dim of scores
S = 2 * H  # 2048
P = 128

NHT = H // P  # 8
NDT = D // P  # 16
NST = S // P  # 16
NQT = TQ // P  # 8
NCH = 512  # PSUM-bank-wide chunk
NSC = S // NCH  # 4
HQ = TQ // 2  # 512, P1 moving width


def build_program() -> bass.Bass:
    nc = bacc.Bacc(
        "TRN2",
        target_bir_lowering=False,
        debug=False,
        dynamic_dma_scratch_size=2048,
    )
    inpT = nc.declare_dram_parameter("inpT", [P, NHT, TQ], FP16, isOutput=False)
    wt = nc.declare_dram_parameter("wt", [P, NDT, NHT, P], FP16, isOutput=False)
    encb = nc.declare_dram_parameter("encb", [NDT, P, S], FP16, isOutput=False)
    bvec = nc.declare_dram_parameter("bvec", [P, NDT], F32, isOutput=False)
    out = nc.declare_dram_parameter("out", [TQ, S], F32, isOutput=True)

    with tile.TileContext(nc) as tc:
        with (
            tc.tile_pool(name="const", bufs=1) as constp,
            tc.tile_pool(name="inp", bufs=1) as inpp,
            tc.tile_pool(name="wtp", bufs=1) as wtp,
            tc.tile_pool(name="projp", bufs=1) as projp,
            tc.tile_pool(name="ep", bufs=2) as ep,
            tc.tile_pool(name="etp", bufs=2) as etp,
            tc.tile_pool(name="outp", bufs=4) as outp,
            tc.tile_pool(name="statp", bufs=3) as statp,
            tc.tile_pool(name="ps_sc", bufs=1, space="PSUM") as ps_sc,
            tc.tile_pool(name="ps_tp", bufs=2, space="PSUM") as ps_tp,
            tc.tile_pool(name="ps_out", bufs=2, space="PSUM") as ps_out,
        ):
            identf = constp.tile([P, P], F32)
            make_identity(nc, identf[:])
            identb = constp.tile([P, P], FP16)
            nc.vector.tensor_copy(identb[:], identf[:])
            bias_sb = constp.tile([P, NDT], F32)
            nc.sync.dma_start(out=bias_sb[:], in_=bvec[:])

            # front loads: sync = wt0, inp(ht 0-3), wt1-3, inp(ht 4-7),
            # wt4-15; scalar = enc 0-15. Contiguous partition rows
            # (host-pretransposed) keep both queues near peak rate.
            inp = inpp.tile([P, NHT, TQ], FP16)
            wt_sb = wtp.tile([P, NDT, NHT, P], FP16)
            # critical-path order: first wt quarter + first inp quarters
            # feed P1 dt0-3; ht 6-7 land via the scalar queue before enc.
            nc.sync.dma_start(out=wt_sb[:, 0:1], in_=wt[:, 0:1])
            nc.sync.dma_start(out=inp[:, 0:2, :], in_=inpT[:, 0:2, :])
            nc.sync.dma_start(out=inp[:, 2:4, :], in_=inpT[:, 2:4, :])
            nc.sync.dma_start(out=wt_sb[:, 1:4], in_=wt[:, 1:4])
            nc.scalar.dma_start(out=inp[:, 4:6, :], in_=inpT[:, 4:6, :])
            nc.scalar.dma_start(out=inp[:, 6:NHT, :], in_=inpT[:, 6:NHT, :])
            nc.sync.dma_start(out=wt_sb[:, 4:10], in_=wt[:, 4:10])
            nc.sync.dma_start(out=wt_sb[:, 10:16], in_=wt[:, 10:16])
            enc_t = constp.tile([P, NDT, S], FP16)
            for g in range(4):
                nc.scalar.dma_start(
                    out=enc_t[:, 4 * g : 4 * g + 4, :],
                    in_=encb[4 * g : 4 * g + 4].transpose([1, 0, 2]),
                )
            enc_sb = [enc_t[:, dt_, :] for dt_ in range(NDT)]

            projT = projp.tile([P, NDT, TQ], FP16)
            scs = []
            for c in range(NSC):
                sc_c = ps_sc.tile([P, NCH], F32, name=f"sc{c}", tag=f"sc{c}")
                scs.append(sc_c)

            # ---- front: P1 (dt-outer, both q-halves) ----
            for dt_ in range(NDT):
                for h in range(2):
                    qs = slice(h * HQ, (h + 1) * HQ)
                    pp = ps_out.tile([P, NCH], F32, tag="po")
                    for ht in range(NHT):
                        nc.tensor.matmul(
                            pp[:],
                            wt_sb[:, dt_, ht, :],
                            inp[:, ht, qs],
                            start=(ht == 0),
                            stop=(ht == NHT - 1),
                        )
                    nc.vector.tensor_scalar_add(
                        projT[:, dt_, qs], pp[:], bias_sb[:, dt_ : dt_ + 1]
                    )

            st = statp.tile([P, 12], F32, tag="st")


            def emit_scores(qt, st_):
                """scores for q-tile qt, c-outer; chunk maxes onto st_[0:4].
                Each 512-chunk has its own PSUM tile so chunk c+1's
                accumulation start does not WAR-wait on chunk c's readers."""
                q0 = qt * P
                for c in range(NSC):
                    cs = slice(c * NCH, (c + 1) * NCH)
                    for dt_ in range(NDT):
                        nc.tensor.matmul(
                            scs[c][:],
                            projT[:, dt_, q0 : q0 + P],
                            enc_sb[dt_][:, cs],
                            start=(dt_ == 0),
                            stop=(dt_ == NDT - 1),
                        )
                    nc.vector.tensor_reduce(
                        st_[:, c : c + 1], scs[c][:], axis=AX.X, op=ALU.max
                    )

            def emit_out(ET, st_, qt):
                for c in range(NSC):
                    cs = slice(c * NCH, (c + 1) * NCH)
                    po = ps_out.tile([P, NCH], F32, tag="po")
                    for s_ in range(NST):
                        nc.tensor.matmul(
                            po[:],
                            ET[:, s_, :],
                            enc_sb[s_][:, cs],
                            start=(s_ == 0),
                            stop=(s_ == NST - 1),
                        )
                    ob = outp.tile([P, NCH], F32, tag="ob")
                    nc.vector.tensor_scalar_mul(ob[:], po[:], st_[:, 6:7])
                    nc.sync.dma_start(
                        out=out[qt * P : (qt + 1) * P, cs], in_=ob[:]
                    )

            # scores(0) c-outer with chunk maxes, like the steady loop
            emit_scores(0, st)
            nc.vector.tensor_reduce(
                st[:, 4:5], st[:, 0:4], axis=AX.X, op=ALU.max, negate=True
            )

            # ---- steady loop ----
            # PE order: [P1 sc(0)] transp(0) sc(1) | out(0) transp(1) sc(2)
            #           | out(1) ... | out(6) transp(7) | out(7).
            # exp(qt) runs on ACT during out(qt-1); scores(qt+1) only
            # needs the sc PSUM back after exp(qt) -- no PE stall.
            def emit_softmax_transp(st_):
                """Exp and transposes per 512-chunk: the PE resumes after
                one chunk's exp, and scores(qt+1) chunk c only waits for
                exp chunk c. Partial sums in st_[:, 8:12], combined."""
                E = ep.tile([P, S], FP16, tag="E")
                ET = etp.tile([P, NST, P], FP16, tag="ET")
                for c in range(NSC):
                    cs = slice(c * NCH, (c + 1) * NCH)
                    nc.scalar.activation(
                        E[:, cs],
                        scs[c][:],
                        AF.Exp,
                        bias=st_[:, 4:5],
                        scale=1.0,
                        accum_out=st_[:, 8 + c : 9 + c],
                    )
                    for s_ in range(4 * c, 4 * c + 4):
                        tp = ps_tp.tile([P, P], FP16, tag="tp")
                        nc.tensor.transpose(
                            tp[:], E[:, s_ * P : (s_ + 1) * P], identb[:]
                        )
                        nc.vector.tensor_copy(ET[:, s_, :], tp[:])
                nc.vector.tensor_reduce(
                    st_[:, 5:6], st_[:, 8:12], axis=AX.X, op=ALU.add
                )
                nc.vector.reciprocal(st_[:, 6:7], st_[:, 5:6])
                return ET

            prev = None  # (ET, st, qt) pending out()
            for qt in range(NQT):
                if prev is not None:
                    emit_out(*prev)
                ET = emit_softmax_transp(st)
                cur_st = st
                if qt + 1 < NQT:
                    st = statp.tile([P, 12], F32, tag="st")
                    emit_scores(qt + 1, st)
                    nc.vector.tensor_reduce(
                        st[:, 4:5], st[:, 0:4], axis=AX.X, op=ALU.max, negate=True
                    )
                prev = (ET, cur_st, qt)
            emit_out(*prev)

    nc.compile()
    return nc


_NC_CACHE = {}


def _get_program() -> bass.Bass:
    if "p" not in _NC_CACHE:
        _NC_CACHE["p"] = build_program()
    return _NC_CACHE["p"]


def _prep_in_maps(input, encoder_output, W, b):
    input = np.ascontiguousarray(input, dtype=np.float32)
    W = np.ascontiguousarray(W, dtype=np.float32)
    b = np.ascontiguousarray(b, dtype=np.float32)

    # inpT[b, hp, ht, q] = input[b, q, ht*P+hp]
    inpT = np.ascontiguousarray(
        input.transpose(0, 2, 1).reshape(B, NHT, P, TQ).transpose(0, 2, 1, 3)
    ).astype(np.float16)
    # wt[hp, dt, ht, dj] = W[dt*P+dj, ht*P+hp]
    wt = np.ascontiguousarray(
        W.reshape(NDT, P, NHT, P).transpose(3, 0, 2, 1)
    ).astype(np.float16)
    bvec = np.ascontiguousarray(b.reshape(NDT, P).T)  # [P, NDT]
    encb = np.ascontiguousarray(encoder_output).astype(np.float16).reshape(B, NDT, P, S)

    return [
        {"inpT": inpT[i], "wt": wt, "encb": encb[i], "bvec": bvec}
        for i in range(B)
    ]


def run(input, encoder_output, W, b, trace=False):
    nc = _get_program()
    in_maps = _prep_in_maps(input, encoder_output, W, b)
    res = run_bass_kernel_spmd(nc, in_maps, list(range(B)), trace=trace)
    out = np.stack([np.asarray(res.results[i]["out"]) for i in range(B)])
    return out, res


def kernel(input, encoder_output, W, b):
    out, _ = run(input, encoder_output, W, b, trace=False)
    return out
